# revision 7
# baseline (speedup 1.0000x reference)
"""CRF negative-log-likelihood loss kernel for 8 Trainium2 NeuronCores.

Full inputs in, full (scalar) output out. The 256-row batch is split into 4
pairs of cores (64 rows per pair). Within a pair, one core runs the FORWARD
exp-space recursion over time steps 0..255 and the other runs the BACKWARD
recursion over steps 511..256 (same SPMD program: the backward core simply
receives time-reversed emissions and the transposed transition matrix), so the
serial per-step chain is 256 steps instead of 511:

    fwd:  V_k = e_k .* S_k ; S_{k+1} = E^T V_k      (E = exp(trans), bf16)
    bwd:  identical with e'_s = e_{511-s}, E' = E^T

The pair meets in the middle: Z_b = sum_j Vfwd_255[j,b] * Sbwd_256[j,b],
combined on the host (tiny [128,64] dot per pair) along with per-core scalar
numerator partials. A constant per-step normalizer C (ln(128*sqrt(e))) is
folded into exp(emissions) on the ACT engine and corrected at the end.

The gold-path numerator (emission picks + transition pairs) runs on device:
per (row, 128-step chunk) task, one GpSimd scatter drops both one-hots into a
[oh(130)|sh(130)|em(128)] gapped emission tile (emissions arrive from the
host pre-cast to bf16 in this layout plus the chain layout [tag, t*64+b]),
then one PSUM-accumulated [128,258] S|Q matmul per task. Tasks are paced one
per two chain steps by a tiny DVE index-copy emitted in the chain's
instruction stream, so scatters/matmuls fill engine idle slots instead of
bursting and stalling the in-order PE queue.
"""

import numpy as np

B_TOT, L, T = 256, 512, 128
NCORES = 8
BP = 64                        # batch rows per core pair
K = L // 2                     # 256 chain steps per core
NTASK = 128                    # numerator tasks per core (64 rows x 2 chunks)
TW = 388                       # task region width: oh(130) | sh(130) | em(128)
C_BIAS = 5.354                 # per-step normalizer (nats)

_CACHE = {}


def _build():
    import concourse.bacc as bacc
    import concourse.tile as tile
    import concourse.mybir as mybir

    dt = mybir.dt
    alu = mybir.AluOpType
    actf = mybir.ActivationFunctionType
    f32 = dt.float32
    bf16 = dt.bfloat16

    nc = bacc.Bacc("TRN2", target_bir_lowering=False, debug=False,
                   num_devices=NCORES)

    emA_d = nc.dram_tensor("em_a", [T, K * BP], bf16, kind="ExternalInput")
    emB_d = nc.dram_tensor("em_b", [T, NTASK * TW], bf16, kind="ExternalInput")
    # packed f32 smalls: trans(0:128) ident(128:256) iota(256:384)
    # stcol(384) strow(row0 @ 400:528)
    pack_d = nc.dram_tensor("pack_f32", [T, 528], f32, kind="ExternalInput")
    # packed int16 scatter indices: col 2j = tag, col 2j+1 = 130 + sh_tag
    pidx_d = nc.dram_tensor("pack_i16", [T, 2 * NTASK], dt.int16,
                            kind="ExternalInput")
    tagsc0_d = nc.dram_tensor("tagsc0", [BP, 1], f32, kind="ExternalInput")
    out_d = nc.dram_tensor("out", [T, 132], f32, kind="ExternalOutput")

    # graded emA pieces: small first so the chain starts early
    PIECES_A = [256, 768, 1024, 2048, 4096, 4096, 4096]
    NPB = 16
    WBT = NTASK // NPB                 # 8 tasks per emB piece

    with tile.TileContext(nc) as tc:
        with (
            tc.tile_pool(name="persist", bufs=1) as pp,
            tc.tile_pool(name="idxp", bufs=4) as ixp,
            tc.tile_pool(name="pchain", bufs=3) as pcp,
            tc.tile_pool(name="upsum", bufs=2, space="PSUM") as up,
            tc.tile_pool(name="spsum", bufs=1, space="PSUM") as sp,
        ):
            # ---- persistent tiles ----
            emA_sb = pp.tile([T, K * BP], bf16)        # 32KB/part
            exp_em = pp.tile([T, K * BP], bf16)        # 32KB/part
            emB_sb = pp.tile([T, NTASK * TW], bf16)    # 97KB/part
            pack_sb = pp.tile([T, 528], f32)
            pidx_sb = pp.tile([T, 2 * NTASK], dt.int16)
            tagsc0 = pp.tile([BP, 1], f32)
            E_bf = pp.tile([T, T], bf16)
            e_init = pp.tile([T, 1], f32)
            st_bc = pp.tile([BP, T], f32)
            scat2 = pp.tile([T, 2], bf16)
            cbias = pp.tile([T, 1], f32)
            g_sb = pp.tile([T, 4], f32)
            junk = pp.tile([T, T], f32)
            junk2 = pp.tile([T, T], f32)
            vlast = pp.tile([T, BP], f32)
            s_sb = pp.tile([T, BP], f32)

            trans_sb = pack_sb[:, 0:128]
            id_sb = pack_sb[:, 128:256]
            iota_sb = pack_sb[0:BP, 256:384]
            stcol_sb = pack_sb[:, 384:385]
            strow_sb = pack_sb[0:1, 400:528]

            # ---- DMAs: sync queue gates the chain, gpsimd queue the tasks --
            nc.sync.dma_start(pack_sb[:], pack_d[:, :])
            off = 0
            for w in PIECES_A:
                nc.sync.dma_start(emA_sb[:, off:off + w], emA_d[:, off:off + w])
                off += w
            nc.gpsimd.dma_start(pidx_sb[:], pidx_d[:, :])
            nc.gpsimd.dma_start(tagsc0[:], tagsc0_d[:, :])
            for i in range(NPB):
                nc.gpsimd.dma_start(
                    emB_sb[:, i * WBT * TW:(i + 1) * WBT * TW],
                    emB_d[:, i * WBT * TW:(i + 1) * WBT * TW])

            nc.vector.memset(cbias[:], -C_BIAS)
            nc.vector.memset(g_sb[:], 0.0)
            nc.vector.memset(scat2[:], 1.0)
            nc.scalar.activation(E_bf[:], trans_sb, actf.Exp)
            nc.scalar.activation(e_init[:], stcol_sb, actf.Exp)
            nc.gpsimd.partition_broadcast(st_bc[:], strow_sb)
            off = 0
            for w in PIECES_A:
                nc.scalar.activation(exp_em[:, off:off + w],
                                     emA_sb[:, off:off + w],
                                     actf.Exp, bias=cbias[:])
                off += w

            # ---- numerator task machinery ----
            sq_psum = sp.tile([T, 258], f32)
            idx_tiles = {}

            def emit_copy(c):
                # DVE idx copy (4 tasks per batch) paces the scatters to the
                # chain position, one batch ahead of their matmuls
                idx4 = ixp.tile([T, 8], dt.int16, name="idx4", tag="ix",
                                bufs=4)
                nc.vector.tensor_copy(idx4[:], pidx_sb[:, 8 * c:8 * c + 8])
                idx_tiles[c] = idx4

            def emit_task(j):
                idx4 = idx_tiles[j // 4]
                q = 2 * (j % 4)
                nc.gpsimd.local_scatter(emB_sb[:, j * TW:j * TW + 260],
                                        scat2[:], idx4[:, q:q + 2],
                                        channels=T, num_elems=260, num_idxs=2)
                nc.tensor.matmul(sq_psum[:], emB_sb[:, j * TW:j * TW + 128],
                                 emB_sb[:, j * TW + 130:(j + 1) * TW],
                                 start=(j == 0), stop=(j == NTASK - 1),
                                 skip_group_check=True)

            # ---- chain ----
            emit_copy(0)
            p_prev = pcp.tile([T, BP], bf16, name="p_t")
            nc.vector.tensor_scalar(p_prev[:], exp_em[:, 0:BP], e_init[:], None,
                                    op0=alu.mult)
            for k in range(1, K):
                u_ps = up.tile([T, BP], f32, name="u_ps")
                nc.tensor.matmul(u_ps[:], E_bf[:], p_prev[:], start=True,
                                 stop=True)
                p_cur = pcp.tile([T, BP], bf16, name="p_t")
                nc.vector.tensor_mul(p_cur[:], u_ps[:],
                                     exp_em[:, k * BP:(k + 1) * BP])
                p_prev = p_cur
                if k % 8 == 0 and k // 8 < NTASK // 4:
                    emit_copy(k // 8)
                if k % 2 == 1 and k // 2 < NTASK:
                    emit_task(k // 2)

            # V_255 (f32 copy) and S_256 = E^T V_255
            nc.scalar.activation(vlast[:], p_prev[:], actf.Copy)
            s_ps = up.tile([T, BP], f32, name="s_ps")
            nc.tensor.matmul(s_ps[:], E_bf[:], p_prev[:], start=True, stop=True)
            nc.scalar.activation(s_sb[:], s_ps[:], actf.Copy)

            # ---- finale: numerator partials into g_sb columns ----
            nc.vector.scalar_tensor_tensor(
                junk[:], sq_psum[:, 0:T], 1.0, trans_sb,
                op0=alu.mult, op1=alu.mult, accum_out=g_sb[:, 0:1])
            nc.vector.scalar_tensor_tensor(
                junk2[:], sq_psum[:, 130:258], 1.0, id_sb,
                op0=alu.mult, op1=alu.mult, accum_out=g_sb[:, 1:2])
            nc.vector.scalar_tensor_tensor(
                junk2[0:BP, :], iota_sb, tagsc0[:], st_bc[:],
                op0=alu.is_equal, op1=alu.mult, accum_out=g_sb[0:BP, 2:3])

            nc.sync.dma_start(out_d[:, 0:BP], vlast[:])
            nc.sync.dma_start(out_d[:, BP:2 * BP], s_sb[:])
            nc.sync.dma_start(out_d[:, 2 * BP:132], g_sb[:])

    nc.compile()
    return nc


def get_nc():
    if "nc" not in _CACHE:
        _CACHE["nc"] = _build()
    return _CACHE["nc"]


def make_in_maps(emissions, tags, start_transitions, end_transitions,
                 transitions):
    import ml_dtypes
    bf = ml_dtypes.bfloat16
    em = np.asarray(emissions, dtype=np.float32)
    tg = np.asarray(tags, dtype=np.int64)
    tr = np.asarray(transitions, dtype=np.float32)
    st = np.asarray(start_transitions, dtype=np.float32)
    en = np.asarray(end_transitions, dtype=np.float32)
    iota = np.tile(np.arange(T, dtype=np.float32), (T, 1))
    ident = np.eye(T, dtype=np.float32)

    in_maps = []
    for core in range(NCORES):
        pair = core // 2
        fwd = (core % 2 == 0)
        rows = slice(pair * BP, (pair + 1) * BP)
        em_c = em[rows]
        tg_c = tg[rows]
        if fwd:
            em_s = em_c[:, :K, :]
            tg_s = tg_c[:, :K]
            tg_sh = np.concatenate([tg_c[:, 1:K], tg_c[:, K:K + 1]], axis=1)
            tg0 = tg_c[:, 0]
            stvec, trans_core = st, tr
        else:
            em_s = em_c[:, L - 1:K - 1:-1, :]
            tg_s = tg_c[:, L - 1:K - 1:-1]
            tg_sh = np.concatenate(
                [tg_c[:, L - 2:K - 1:-1],
                 np.full((BP, 1), 128, np.int64)], axis=1)
            tg0 = tg_c[:, L - 1]
            stvec, trans_core = en, np.ascontiguousarray(tr.T)
        # chain layout [tag, t*64+b]
        emA = np.ascontiguousarray(
            em_s.transpose(2, 1, 0).reshape(T, K * BP)).astype(bf)
        # task layout: [tlo, idx*388 + (260 + g)], idx = c*64 + b
        emB3 = np.zeros((T, NTASK, TW), dtype=bf)
        emB3[:, :, 260:TW] = em_s.reshape(BP, 2, 128, T).transpose(
            2, 1, 0, 3).reshape(T, NTASK, T).astype(bf)
        # packed scatter indices: (tag, 130 + sh_tag) per task column pair
        pidx = np.empty((T, 2 * NTASK), np.int16)
        pidx[:, 0::2] = tg_s.reshape(BP, 2, 128).transpose(2, 1, 0).reshape(
            T, NTASK)
        pidx[:, 1::2] = 130 + tg_sh.reshape(BP, 2, 128).transpose(
            2, 1, 0).reshape(T, NTASK)
        # packed f32 smalls
        pack = np.zeros((T, 528), np.float32)
        pack[:, 0:128] = trans_core
        pack[:, 128:256] = ident
        pack[0:BP, 256:384] = iota[0:BP]
        pack[:, 384] = stvec
        pack[0, 400:528] = stvec
        in_maps.append({
            "em_a": emA,
            "em_b": np.ascontiguousarray(emB3.reshape(T, NTASK * TW)),
            "pack_f32": pack,
            "pack_i16": pidx,
            "tagsc0": np.ascontiguousarray(
                tg0.reshape(BP, 1).astype(np.float32)),
        })
    return in_maps


def kernel(emissions, tags, mask, start_transitions, end_transitions,
           transitions):
    from concourse.bass_utils import run_bass_kernel_spmd

    nc = get_nc()
    in_maps = make_in_maps(emissions, tags, start_transitions,
                           end_transitions, transitions)
    res = run_bass_kernel_spmd(nc, in_maps, core_ids=list(range(NCORES)),
                               trace=bool(_CACHE.get("trace", False)))
    _CACHE["last_result"] = res
    outs = [np.asarray(r["out"], dtype=np.float64) for r in res.results]
    num_total = sum(o[:, 128:131].sum() for o in outs)
    lnZ_sum = 0.0
    for pair in range(NCORES // 2):
        vf = outs[2 * pair][:, 0:BP]
        sb = outs[2 * pair + 1][:, BP:2 * BP]
        Z = (vf * sb).sum(axis=0)
        lnZ_sum += (np.log(Z) + L * C_BIAS).sum()
    return np.float32(num_total - lnZ_sum)


# revision 8
# speedup vs baseline: 1.1421x; 1.1421x over previous
"""CRF negative-log-likelihood loss kernel for 8 Trainium2 NeuronCores.

Full inputs in, full (scalar) output out. The 256-row batch is split into 4
pairs of cores (64 rows per pair). Within a pair, one core runs the FORWARD
exp-space recursion over time steps 0..255 and the other runs the BACKWARD
recursion over steps 511..256 (same SPMD program: the backward core simply
receives time-reversed emissions and the transposed transition matrix), so the
serial per-step chain is 256 steps instead of 511:

    fwd:  V_k = e_k .* S_k ; S_{k+1} = E^T V_k      (E = exp(trans), bf16)
    bwd:  identical with e'_s = e_{511-s}, E' = E^T

The pair meets in the middle: Z_b = sum_j Vfwd_255[j,b] * Sbwd_256[j,b],
combined on the host (tiny [128,64] dot per pair) along with per-core scalar
numerator partials. A constant per-step normalizer C (ln(128*sqrt(e))) is
folded into exp(emissions) on the ACT engine and corrected at the end.

The gold-path numerator (emission picks + transition pairs) runs on device:
per (row, 128-step chunk) task, one GpSimd scatter drops both one-hots into a
[oh(130)|sh(130)|em(128)] gapped emission tile (emissions arrive from the
host pre-cast to bf16 in this layout plus the chain layout [tag, t*64+b]),
then one PSUM-accumulated [128,258] S|Q matmul per task. Tasks are paced one
per two chain steps by a tiny DVE index-copy emitted in the chain's
instruction stream, so scatters/matmuls fill engine idle slots instead of
bursting and stalling the in-order PE queue.
"""

import numpy as np

B_TOT, L, T = 256, 512, 128
NCORES = 8
BP = 64                        # batch rows per core pair
K = L // 2                     # 256 chain steps per core
NTASK = 128                    # numerator tasks per core (64 rows x 2 chunks)
TW = 388                       # task region width: oh(130) | sh(130) | em(128)
C_BIAS = 5.354                 # per-step normalizer (nats)

_CACHE = {}


def _build():
    import concourse.bacc as bacc
    import concourse.tile as tile
    import concourse.mybir as mybir

    dt = mybir.dt
    alu = mybir.AluOpType
    actf = mybir.ActivationFunctionType
    f32 = dt.float32
    bf16 = dt.bfloat16

    nc = bacc.Bacc("TRN2", target_bir_lowering=False, debug=False,
                   num_devices=NCORES)

    emA_d = nc.dram_tensor("em_a", [T, K * BP], bf16, kind="ExternalInput")
    emB_d = nc.dram_tensor("em_b", [T, NTASK * TW], bf16, kind="ExternalInput")
    # packed f32 smalls: trans(0:128) ident(128:256) iota(256:384)
    # stcol(384) strow(row0 @ 400:528)
    pack_d = nc.dram_tensor("pack_f32", [T, 528], f32, kind="ExternalInput")
    # packed int16 scatter indices: col 2j = tag, col 2j+1 = 130 + sh_tag
    pidx_d = nc.dram_tensor("pack_i16", [T, 2 * NTASK], dt.int16,
                            kind="ExternalInput")
    tagsc0_d = nc.dram_tensor("tagsc0", [BP, 1], f32, kind="ExternalInput")
    out_d = nc.dram_tensor("out", [T, 132], f32, kind="ExternalOutput")

    # graded emA pieces: small first so the chain starts early
    PIECES_A = [256, 768, 1024, 2048, 4096, 4096, 4096]
    NPB = 16
    WBT = NTASK // NPB                 # 8 tasks per emB piece

    with tile.TileContext(nc) as tc:
        with (
            tc.tile_pool(name="persist", bufs=1) as pp,
            tc.tile_pool(name="idxp", bufs=4) as ixp,
            tc.tile_pool(name="pchain", bufs=3) as pcp,
            tc.tile_pool(name="upsum", bufs=2, space="PSUM") as up,
            tc.tile_pool(name="spsum", bufs=1, space="PSUM") as sp,
        ):
            # ---- persistent tiles ----
            emA_sb = pp.tile([T, K * BP], bf16)        # 32KB/part
            exp_em = pp.tile([T, K * BP], bf16)        # 32KB/part
            emB_sb = pp.tile([T, NTASK * TW], bf16)    # 97KB/part
            pack_sb = pp.tile([T, 528], f32)
            pidx_sb = pp.tile([T, 2 * NTASK], dt.int16)
            tagsc0 = pp.tile([BP, 1], f32)
            E_bf = pp.tile([T, T], bf16)
            e_init = pp.tile([T, 1], f32)
            st_bc = pp.tile([BP, T], f32)
            scat2 = pp.tile([T, 2], bf16)
            cbias = pp.tile([T, 1], f32)
            g_sb = pp.tile([T, 4], f32)
            junk = pp.tile([T, T], f32)
            junk2 = pp.tile([T, T], f32)
            vlast = pp.tile([T, BP], f32)
            s_sb = pp.tile([T, BP], f32)

            trans_sb = pack_sb[:, 0:128]
            id_sb = pack_sb[:, 128:256]
            iota_sb = pack_sb[0:BP, 256:384]
            stcol_sb = pack_sb[:, 384:385]
            strow_sb = pack_sb[0:1, 400:528]

            # ---- DMAs: sync queue gates the chain, gpsimd queue the tasks --
            nc.sync.dma_start(pack_sb[:], pack_d[:, :])
            off = 0
            for w in PIECES_A:
                nc.sync.dma_start(emA_sb[:, off:off + w], emA_d[:, off:off + w])
                off += w
            nc.gpsimd.dma_start(pidx_sb[:], pidx_d[:, :])
            nc.gpsimd.dma_start(tagsc0[:], tagsc0_d[:, :])
            for i in range(NPB):
                nc.gpsimd.dma_start(
                    emB_sb[:, i * WBT * TW:(i + 1) * WBT * TW],
                    emB_d[:, i * WBT * TW:(i + 1) * WBT * TW])

            nc.vector.memset(cbias[:], -C_BIAS)
            nc.vector.memset(g_sb[:], 0.0)
            nc.vector.memset(scat2[:], 1.0)
            nc.scalar.activation(E_bf[:], trans_sb, actf.Exp)
            nc.scalar.activation(e_init[:], stcol_sb, actf.Exp)
            nc.gpsimd.partition_broadcast(st_bc[:], strow_sb)
            off = 0
            for w in PIECES_A:
                nc.scalar.activation(exp_em[:, off:off + w],
                                     emA_sb[:, off:off + w],
                                     actf.Exp, bias=cbias[:])
                off += w

            # ---- numerator task machinery ----
            sq_psum = sp.tile([T, 258], f32)
            idx_tiles = {}

            def emit_copy(c):
                # DVE idx copy (4 tasks per batch) paces the scatters to the
                # chain position, one batch ahead of their matmuls
                idx4 = ixp.tile([T, 8], dt.int16, name="idx4", tag="ix",
                                bufs=4)
                nc.vector.tensor_copy(idx4[:], pidx_sb[:, 8 * c:8 * c + 8])
                idx_tiles[c] = idx4

            def emit_task(j):
                idx4 = ixp.tile([T, 2], dt.int16, name="idxj", tag="ix",
                                bufs=4)
                nc.vector.tensor_copy(idx4[:], pidx_sb[:, 2 * j:2 * j + 2])
                q = 0
                nc.gpsimd.local_scatter(emB_sb[:, j * TW:j * TW + 260],
                                        scat2[:], idx4[:, q:q + 2],
                                        channels=T, num_elems=260, num_idxs=2)
                nc.tensor.matmul(sq_psum[:], emB_sb[:, j * TW:j * TW + 128],
                                 emB_sb[:, j * TW + 130:(j + 1) * TW],
                                 start=(j == 0), stop=(j == NTASK - 1),
                                 skip_group_check=True)

            # ---- chain ----
            p_prev = pcp.tile([T, BP], bf16, name="p_t")
            nc.vector.tensor_scalar(p_prev[:], exp_em[:, 0:BP], e_init[:], None,
                                    op0=alu.mult)
            for k in range(1, K):
                u_ps = up.tile([T, BP], f32, name="u_ps")
                nc.tensor.matmul(u_ps[:], E_bf[:], p_prev[:], start=True,
                                 stop=True)
                p_cur = pcp.tile([T, BP], bf16, name="p_t")
                nc.vector.tensor_mul(p_cur[:], u_ps[:],
                                     exp_em[:, k * BP:(k + 1) * BP])
                p_prev = p_cur
                if k % 2 == 1 and k // 2 < NTASK:
                    emit_task(k // 2)

            # V_255 (f32 copy) and S_256 = E^T V_255
            nc.scalar.activation(vlast[:], p_prev[:], actf.Copy)
            s_ps = up.tile([T, BP], f32, name="s_ps")
            nc.tensor.matmul(s_ps[:], E_bf[:], p_prev[:], start=True, stop=True)
            nc.scalar.activation(s_sb[:], s_ps[:], actf.Copy)

            # ---- finale: numerator partials into g_sb columns ----
            nc.vector.scalar_tensor_tensor(
                junk[:], sq_psum[:, 0:T], 1.0, trans_sb,
                op0=alu.mult, op1=alu.mult, accum_out=g_sb[:, 0:1])
            nc.vector.scalar_tensor_tensor(
                junk2[:], sq_psum[:, 130:258], 1.0, id_sb,
                op0=alu.mult, op1=alu.mult, accum_out=g_sb[:, 1:2])
            nc.vector.scalar_tensor_tensor(
                junk2[0:BP, :], iota_sb, tagsc0[:], st_bc[:],
                op0=alu.is_equal, op1=alu.mult, accum_out=g_sb[0:BP, 2:3])

            nc.sync.dma_start(out_d[:, 0:BP], vlast[:])
            nc.sync.dma_start(out_d[:, BP:2 * BP], s_sb[:])
            nc.sync.dma_start(out_d[:, 2 * BP:132], g_sb[:])

    nc.compile()
    return nc


def get_nc():
    if "nc" not in _CACHE:
        _CACHE["nc"] = _build()
    return _CACHE["nc"]


def make_in_maps(emissions, tags, start_transitions, end_transitions,
                 transitions):
    import ml_dtypes
    bf = ml_dtypes.bfloat16
    em = np.asarray(emissions, dtype=np.float32)
    tg = np.asarray(tags, dtype=np.int64)
    tr = np.asarray(transitions, dtype=np.float32)
    st = np.asarray(start_transitions, dtype=np.float32)
    en = np.asarray(end_transitions, dtype=np.float32)
    iota = np.tile(np.arange(T, dtype=np.float32), (T, 1))
    ident = np.eye(T, dtype=np.float32)

    in_maps = []
    for core in range(NCORES):
        pair = core // 2
        fwd = (core % 2 == 0)
        rows = slice(pair * BP, (pair + 1) * BP)
        em_c = em[rows]
        tg_c = tg[rows]
        if fwd:
            em_s = em_c[:, :K, :]
            tg_s = tg_c[:, :K]
            tg_sh = np.concatenate([tg_c[:, 1:K], tg_c[:, K:K + 1]], axis=1)
            tg0 = tg_c[:, 0]
            stvec, trans_core = st, tr
        else:
            em_s = em_c[:, L - 1:K - 1:-1, :]
            tg_s = tg_c[:, L - 1:K - 1:-1]
            tg_sh = np.concatenate(
                [tg_c[:, L - 2:K - 1:-1],
                 np.full((BP, 1), 128, np.int64)], axis=1)
            tg0 = tg_c[:, L - 1]
            stvec, trans_core = en, np.ascontiguousarray(tr.T)
        # chain layout [tag, t*64+b]
        emA = np.ascontiguousarray(
            em_s.transpose(2, 1, 0).reshape(T, K * BP)).astype(bf)
        # task layout: [tlo, idx*388 + (260 + g)], idx = c*64 + b
        emB3 = np.zeros((T, NTASK, TW), dtype=bf)
        emB3[:, :, 260:TW] = em_s.reshape(BP, 2, 128, T).transpose(
            2, 1, 0, 3).reshape(T, NTASK, T).astype(bf)
        # packed scatter indices: (tag, 130 + sh_tag) per task column pair
        pidx = np.empty((T, 2 * NTASK), np.int16)
        pidx[:, 0::2] = tg_s.reshape(BP, 2, 128).transpose(2, 1, 0).reshape(
            T, NTASK)
        pidx[:, 1::2] = 130 + tg_sh.reshape(BP, 2, 128).transpose(
            2, 1, 0).reshape(T, NTASK)
        # packed f32 smalls
        pack = np.zeros((T, 528), np.float32)
        pack[:, 0:128] = trans_core
        pack[:, 128:256] = ident
        pack[0:BP, 256:384] = iota[0:BP]
        pack[:, 384] = stvec
        pack[0, 400:528] = stvec
        in_maps.append({
            "em_a": emA,
            "em_b": np.ascontiguousarray(emB3.reshape(T, NTASK * TW)),
            "pack_f32": pack,
            "pack_i16": pidx,
            "tagsc0": np.ascontiguousarray(
                tg0.reshape(BP, 1).astype(np.float32)),
        })
    return in_maps


def kernel(emissions, tags, mask, start_transitions, end_transitions,
           transitions):
    from concourse.bass_utils import run_bass_kernel_spmd

    nc = get_nc()
    in_maps = make_in_maps(emissions, tags, start_transitions,
                           end_transitions, transitions)
    res = run_bass_kernel_spmd(nc, in_maps, core_ids=list(range(NCORES)),
                               trace=bool(_CACHE.get("trace", False)))
    _CACHE["last_result"] = res
    outs = [np.asarray(r["out"], dtype=np.float64) for r in res.results]
    num_total = sum(o[:, 128:131].sum() for o in outs)
    lnZ_sum = 0.0
    for pair in range(NCORES // 2):
        vf = outs[2 * pair][:, 0:BP]
        sb = outs[2 * pair + 1][:, BP:2 * BP]
        Z = (vf * sb).sum(axis=0)
        lnZ_sum += (np.log(Z) + L * C_BIAS).sum()
    return np.float32(num_total - lnZ_sum)
